# revision 1
# baseline (speedup 1.0000x reference)
"""Trainium2 Bass kernel for 2D single-level DWT (coif1, symmetric padding).

Input  x: (4, 64, 512, 512) fp32
Output  : (4, 256, 258, 258) fp32  -- per input channel: [cA, cH, cV, cD]

Math: with R_f the banded 258x512 operator of the 1D DWT along an axis
(6-tap filter, stride 2, symmetric boundary folds), the four outputs are
    cA = R_lo X R_lo^T,  cH = R_hi X R_lo^T,
    cV = R_lo X R_hi^T,  cD = R_hi X R_hi^T.

On-device (per image, per core; 32 images per core, pure data-parallel):
  pass 1 (contract over rows r on the PE):   Yt_f[c, kh] = sum_r X[r, c] R_f[kh, r]
     matmul with lhsT = X column-slice (stationary), rhs = R_f^T chunk.
  pass 2 (contract over cols c on the PE):   O_s[kw, kh] = sum_c R_g[kw, c] Yt_f[c, kh]
     matmul with lhsT = R_g^T kw-slice (stationary), rhs = Yt_f chunk.
  Outputs land transposed ([kw, kh]); the host swaps the last two axes.

Matmuls run as float32r (1 cycle/row for N>=256, numerically fp32-width).
"""

import os
import sys

for _p in ("/opt/trn_rl_repo", "/opt/pypackages"):
    if _p not in sys.path:
        sys.path.append(_p)

os.environ.setdefault("JAX_COMPILATION_CACHE_DIR", "/tmp/jax_comp_cache")
os.environ.setdefault("JAX_PERSISTENT_CACHE_MIN_COMPILE_TIME_SECS", "10")

import numpy as np

import concourse.bass as bass
import concourse.bacc as bacc
import concourse.mybir as mybir
from concourse.bass_utils import run_bass_kernel_spmd
from concourse.tile import TileContext

N_CORES = 8
H = W = 512
OUT = 258  # (512 + 6 - 1) // 2
IMGS = 32  # images per core (4*64/8)
F32 = mybir.dt.float32
F32R = mybir.dt.float32r

# pywt coif1 decomposition filters (already flipped: correlation form)
DEC_LO = np.array([-0.01565572813546454, -0.0727326195128539, 0.38486484686420286,
                   0.8525720202122554, 0.3378976624578092, -0.0727326195128539])
DEC_HI = np.array([0.0727326195128539, 0.3378976624578092, -0.8525720202122554,
                   0.38486484686420286, 0.0727326195128539, -0.01565572813546454])
FLEN = 6
PAD = 4
LO_F = DEC_LO[::-1]
HI_F = DEC_HI[::-1]


def _build_R(filt: np.ndarray, n: int = W) -> np.ndarray:
    """Banded [258, 512] operator: out[k] = sum_j filt[j] * x[sym(2k + j - PAD)]."""
    out_len = (n + FLEN - 1) // 2

    def sym(i: int) -> int:
        while i < 0 or i >= n:
            if i < 0:
                i = -i - 1
            if i >= n:
                i = 2 * n - 1 - i
        return i

    R = np.zeros((out_len, n), dtype=np.float64)
    for k in range(out_len):
        for j in range(FLEN):
            R[k, sym(2 * k + j - PAD)] += filt[j]
    return R


def _build_weights() -> np.ndarray:
    """w[p, (f*4+q)*258 + k] = R_f[k, 128q + p], as [128, 8*258] fp32."""
    Rs = [_build_R(LO_F), _build_R(HI_F)]
    tiles = []
    for f in range(2):
        for q in range(4):
            tiles.append(Rs[f][:, 128 * q:128 * (q + 1)].T)
    stacked = np.stack(tiles)  # [8, 128, 258]
    w = np.ascontiguousarray(stacked.transpose(1, 0, 2).reshape(128, 8 * OUT)).astype(np.float32)
    return _round_tf32(w)


def _round_tf32(a: np.ndarray) -> np.ndarray:
    """Round-to-nearest-even to tf32 (10-bit mantissa), keeping fp32 storage."""
    bits = a.astype(np.float32).view(np.uint32)
    bits = (bits + 0xFFF + ((bits >> 13) & 1)) & np.uint32(0xFFFFE000)
    return bits.view(np.float32)


_WEIGHTS = _build_weights()
_MODULE = None
PS1_BUFS = 3
PS2_BUFS = 5
W_RING_SCALAR = False
XPOOL_BUFS = 2
YPOOL_BUFS = 2
SPOOL_BUFS = 2
OUT_SPLIT = 4
IN_SPLIT = 1


def _build_module() -> bass.Bass:
    nc = bacc.Bacc("TRN2", target_bir_lowering=False, debug=False)
    x_in = nc.declare_dram_parameter("x", [IMGS, H, W], F32R, isOutput=False)
    w_in = nc.declare_dram_parameter("w", [128, 8 * OUT], F32R, isOutput=False)
    # device layout: y[i, s, kw, kh] = O_s[kw, kh] (host swaps kh/kw)
    y_out = nc.declare_dram_parameter("y", [IMGS, 4, OUT, OUT], F32, isOutput=True)

    with TileContext(nc) as tc:
        with (
            tc.tile_pool(name="wpool", bufs=1) as wpool,
            tc.tile_pool(name="xpool", bufs=XPOOL_BUFS) as xpool,
            tc.tile_pool(name="ypool", bufs=YPOOL_BUFS) as ypool,
            tc.tile_pool(name="spool", bufs=SPOOL_BUFS) as spool,
            tc.tile_pool(name="psum", bufs=4, space="PSUM") as pspool,
        ):
            Wt = wpool.tile([128, 8 * OUT], F32R)
            # scalar-ring HWDGE so the weight load overlaps the first X load
            (nc.scalar if W_RING_SCALAR else nc.sync).dma_start(out=Wt[:], in_=w_in[:])
            Wr = Wt[:]

            # Tiny PE op consuming the weight DMA so later matmuls depend on
            # it via PE program order (Matmult carries at most one sync wait).
            warm = pspool.tile([1, OUT], F32, tag="ps2", bufs=PS2_BUFS)
            nc.tensor.matmul(warm[:, :], lhsT=Wr[:, 0:1], rhs=Wr[:, 0:OUT],
                             start=True, stop=True)

            def load_x(i):
                # X[p, q*512 + c] = x[i, 128q + p, c]
                X = xpool.tile([128, 4 * W], F32R, tag="X", name=f"X_{i}")
                xi = x_in[i].rearrange("(q p) c -> p q c", p=128)
                Xv = X.rearrange("p (q c) -> p q c", q=4)
                qper = 4 // IN_SPLIT
                for j in range(IN_SPLIT):
                    nc.sync.dma_start(
                        out=Xv[:, j * qper:(j + 1) * qper],
                        in_=xi[:, j * qper:(j + 1) * qper],
                    )
                return X

            ev = 0
            Xnext = load_x(0)
            for i in range(IMGS):
                Xr = Xnext[:]

                # pass 1: Yt[p, (f*4+cc)*258 + kh] = Yt_f[c = 128cc + p, kh]
                Yt = ypool.tile([128, 8 * OUT], F32R, tag="Yt")
                for f in range(2):
                    for cc in range(4):
                        ps = pspool.tile([128, OUT], F32, tag="ps1", bufs=PS1_BUFS)
                        for q in range(4):
                            nc.tensor.matmul(
                                ps[:, :],
                                lhsT=Xr[:, q * W + cc * 128: q * W + (cc + 1) * 128],
                                rhs=Wr[:, (f * 4 + q) * OUT: (f * 4 + q + 1) * OUT],
                                start=(q == 0),
                                stop=(q == 3),
                            )
                        dst = Yt[:, (f * 4 + cc) * OUT: (f * 4 + cc + 1) * OUT]
                        if ev % 2 == 0:
                            nc.scalar.copy(out=dst, in_=ps[:, :])
                        else:
                            nc.vector.tensor_copy(out=dst, in_=ps[:, :])
                        ev += 1
                Ytr = Yt[:]

                # prefetch the next image's input ahead of this image's stores
                # in the sync-ring FIFO
                if i + 1 < IMGS:
                    Xnext = load_x(i + 1)

                # pass 2: STG[p, (s*3+m)*258 + kh] = O_s[kw = 86m + p, kh]
                STG = spool.tile([86, 12 * OUT], F32, tag="STG")
                for g in range(2):
                    for f in range(2):
                        s = f + 2 * g
                        for m in range(3):
                            ps2 = pspool.tile([86, OUT], F32, tag="ps2", bufs=PS2_BUFS)
                            for j, q in enumerate((m, m + 1)):
                                nc.tensor.matmul(
                                    ps2[:, :],
                                    lhsT=Wr[:, (g * 4 + q) * OUT + m * 86:
                                            (g * 4 + q) * OUT + (m + 1) * 86],
                                    rhs=Ytr[:, (f * 4 + q) * OUT: (f * 4 + q + 1) * OUT],
                                    start=(j == 0),
                                    stop=(j == 1),
                                )
                            dst = STG[:, (s * 3 + m) * OUT: (s * 3 + m + 1) * OUT]
                            if ev % 2 == 0:
                                nc.scalar.copy(out=dst, in_=ps2[:, :])
                            else:
                                nc.vector.tensor_copy(out=dst, in_=ps2[:, :])
                            ev += 1

                for s in range(4):
                    nc.sync.dma_start(
                        out=y_out[i, s].rearrange("(m p) k -> p m k", p=86),
                        in_=STG[:, s * 3 * OUT:(s + 1) * 3 * OUT].rearrange(
                            "p (m k) -> p m k", m=3),
                    )
    nc.finalize()
    return nc


def _get_module() -> bass.Bass:
    global _MODULE
    if _MODULE is None:
        _MODULE = _build_module()
    return _MODULE


def kernel(**inputs) -> np.ndarray:
    x = np.asarray(inputs["x"], dtype=np.float32)
    B, C, Hx, Wx = x.shape
    assert (Hx, Wx) == (H, W) and B * C == N_CORES * IMGS
    imgs = x.reshape(B * C, H, W)

    nc = _get_module()
    in_maps = [
        {"x": _round_tf32(imgs[k * IMGS:(k + 1) * IMGS]), "w": _WEIGHTS}
        for k in range(N_CORES)
    ]
    res = run_bass_kernel_spmd(nc, in_maps, list(range(N_CORES))).results

    full = np.concatenate([res[k]["y"] for k in range(N_CORES)], axis=0)
    # device layout is [img, s, kw, kh] -> swap to [img, s, kh, kw]
    full = full.transpose(0, 1, 3, 2)
    return np.ascontiguousarray(full.reshape(B, 4 * C, OUT, OUT)).astype(np.float32)



# revision 5
# speedup vs baseline: 5.4289x; 5.4289x over previous
"""Trainium2 Bass kernel for 2D single-level DWT (coif1, symmetric padding).

Input  x: (4, 64, 512, 512) fp32
Output  : (4, 256, 258, 258) fp32  -- per input channel: [cA, cH, cV, cD]

v2 design (bf16, banded half-blocks, SWDGE stores):
  pass 1 (contract rows r): r is split into half-blocks h in {0,1} of 256
    contiguous rows, each loaded as partition p <- rows (256h + 2p + j),
    j in {0,1} -- every DMA descriptor is 2 contiguous DRAM rows (2 KB).
    The 6-tap band of R_f means half-block h only feeds kh in
    [128h, 128h+130), so each matmul streams only 130 columns; the 2-col
    overlap accumulates via PSUM has_written semantics.
      Yt_f[c, kh] = sum_r X[r, c] R_f[kh, r]
  pass 2 (contract cols c): output rows (g, kw) are packed in uniform
    128-partition chunks of 64 kw x {lo,hi}: chunks start at kw =
    0, 64, 128, 192, 194 (the last overlaps; host keeps only kw 256-257
    from it).  Each chunk's band covers at most 2 c-blocks of 128 -> 1-2
    accumulating matmuls of 258 columns.
      O_{f,g}[kw, kh] = sum_c R_g[kw, c] Yt_f[c, kh]
  All matmuls bf16 (1 cycle/row at any free size, FWL weight loads).
  Loads and stores ride the gpsimd SWDGE ring: HWDGE stores with <128
  partitions serialize onto 2 of 16 SDMA engines; SWDGE spreads all 16.
  Output leaves packed ([i, p, block, kh], bf16); the host unpacks.
"""

import os
import sys

for _p in ("/opt/trn_rl_repo", "/opt/pypackages"):
    if _p not in sys.path:
        sys.path.append(_p)

os.environ.setdefault("JAX_COMPILATION_CACHE_DIR", "/tmp/jax_comp_cache")
os.environ.setdefault("JAX_PERSISTENT_CACHE_MIN_COMPILE_TIME_SECS", "10")

import numpy as np
import ml_dtypes

import concourse.bass as bass
import concourse.bacc as bacc
import concourse.mybir as mybir
from concourse.bass_utils import run_bass_kernel_spmd
from concourse.tile import TileContext

N_CORES = 8
H = W = 512
OUT = 258
IMGS = 32  # images per core (4*64/8)
F32 = mybir.dt.float32
BF16 = mybir.dt.bfloat16
NPBF16 = ml_dtypes.bfloat16

# pywt coif1 decomposition filters, flipped to correlation form
DEC_LO = np.array([-0.01565572813546454, -0.0727326195128539, 0.38486484686420286,
                   0.8525720202122554, 0.3378976624578092, -0.0727326195128539])
DEC_HI = np.array([0.0727326195128539, 0.3378976624578092, -0.8525720202122554,
                   0.38486484686420286, 0.0727326195128539, -0.01565572813546454])
FLEN = 6
PAD = 4
LO_F = DEC_LO[::-1]
HI_F = DEC_HI[::-1]

# pass-2 packed chunks: 64 kw starting at KWSTART[C]; CHUNK_CC[C] = c-blocks
KWSTART = [0, 64, 128, 192, 194]
CHUNK_CC = [[0], [0, 1], [1, 2], [2, 3], [3]]
W2_SLICES = [(C, cc) for C in range(5) for cc in CHUNK_CC[C]]  # 8 slices
# STG block order = device copy order (tA_f0, tB_f0, tA_f1, tB_f1, t5)
STG_BLOCKS = [(0, 0), (0, 1), (0, 2), (0, 3),
              (1, 0), (1, 1), (1, 2), (1, 3),
              (0, 4), (1, 4)]


def _build_R(filt: np.ndarray, n: int = W) -> np.ndarray:
    """Banded [258, 512] operator: out[k] = sum_j filt[j] * x[sym(2k + j - PAD)]."""
    out_len = (n + FLEN - 1) // 2

    def sym(i: int) -> int:
        while i < 0 or i >= n:
            if i < 0:
                i = -i - 1
            if i >= n:
                i = 2 * n - 1 - i
        return i

    R = np.zeros((out_len, n), dtype=np.float64)
    for k in range(out_len):
        for j in range(FLEN):
            R[k, sym(2 * k + j - PAD)] += filt[j]
    return R


_R = [_build_R(LO_F), _build_R(HI_F)]


def _build_w1() -> np.ndarray:
    """w1[p, ((f*2+h)*2+j)*130 + t] = R_f[128h + t, 256h + 2p + j]."""
    w = np.zeros((128, 8, 130), np.float64)
    for f in range(2):
        for h in range(2):
            for j in range(2):
                rows = 256 * h + 2 * np.arange(128) + j
                khs = 128 * h + np.arange(130)
                w[:, (f * 2 + h) * 2 + j, :] = _R[f][np.ix_(khs, rows)].T
    return w.reshape(128, 8 * 130).astype(NPBF16)


def _build_w2() -> np.ndarray:
    """w2[p, s*128 + u] for slice s=(C, cc): R_{u//64}[KWSTART[C] + u%64, 128cc + p]."""
    cols = []
    for C, cc in W2_SLICES:
        w = np.zeros((128, 128), np.float64)
        for u in range(128):
            g, kwo = divmod(u, 64)
            w[:, u] = _R[g][KWSTART[C] + kwo, cc * 128:(cc + 1) * 128]
        cols.append(w)
    return np.concatenate(cols, axis=1).astype(NPBF16)


_W1 = _build_w1()
_W2 = _build_w2()
_MODULE = None
PS1_BUFS = 2
PS2_BUFS = 2
X_BUFS = 3
YT_BUFS = 2
STG_BUFS = 2


def _build_module() -> bass.Bass:
    nc = bacc.Bacc("TRN2", target_bir_lowering=False, debug=False)
    x_in = nc.declare_dram_parameter("x", [IMGS, H, W], BF16, isOutput=False)
    w1_in = nc.declare_dram_parameter("w1", [128, 8 * 130], BF16, isOutput=False)
    w2_in = nc.declare_dram_parameter("w2", [128, 8 * 128], BF16, isOutput=False)
    y_out = nc.declare_dram_parameter("y", [IMGS, 128, 10 * OUT], BF16, isOutput=True)

    with TileContext(nc) as tc:
        with (
            tc.tile_pool(name="wpool", bufs=1) as wpool,
            tc.tile_pool(name="xpool", bufs=X_BUFS) as xpool,
            tc.tile_pool(name="ypool", bufs=YT_BUFS) as ypool,
            tc.tile_pool(name="spool", bufs=STG_BUFS) as spool,
            tc.tile_pool(name="psum", bufs=2, space="PSUM") as pspool,
        ):
            Wt1 = wpool.tile([128, 8 * 130], BF16)
            Wt2 = wpool.tile([128, 8 * 128], BF16)
            nc.sync.dma_start(out=Wt1[:], in_=w1_in[:])
            nc.sync.dma_start(out=Wt2[:], in_=w2_in[:])

            # Tiny PE op consuming both weight DMAs so later matmuls depend
            # on them via PE program order (Matmult carries one sync wait).
            warm = pspool.tile([128, 1024], F32, tag="ps2", bufs=PS2_BUFS,
                               name="warm")
            nc.tensor.matmul(warm[0:1, 0:OUT], lhsT=Wt2[:, 0:1],
                             rhs=Wt1[:, 0:OUT], start=True, stop=True)

            ev = 0

            def copy(dst, src):
                nonlocal ev
                if ev % 2 == 0:
                    nc.scalar.copy(out=dst, in_=src)
                else:
                    nc.vector.tensor_copy(out=dst, in_=src)
                ev += 1

            def load_x(i):
                # X[p, h, j*512 + c] = x[i, 256h + 2p + j, c]
                X = xpool.tile([128, 2, 1024], BF16, tag="X", name=f"X_{i}")
                nc.gpsimd.dma_start(
                    out=X[:],
                    in_=x_in[i].rearrange("(h p j) c -> p h (j c)", h=2, j=2),
                )
                return X

            Xnext = load_x(0)
            for i in range(IMGS):
                Xr = Xnext[:]
                Yt = ypool.tile([128, 8 * OUT], BF16, tag="Yt", name=f"Yt_{i}")

                # pass 1: Yt[p, (f*4+cc)*258 + kh] = Yt_f[c = 128cc + p, kh]
                for ccp in range(2):  # cc pair (2*ccp, 2*ccp+1)
                    ps1 = [pspool.tile([128, 1024], F32, tag="ps1", bufs=PS1_BUFS,
                                       name=f"ps1_{i}_{ccp}_{f}")
                           for f in range(2)]
                    for ci in range(2):
                        cc = 2 * ccp + ci
                        for h in range(2):
                            for j in range(2):
                                lhsT = Xr[:, h, j * 512 + cc * 128:
                                          j * 512 + cc * 128 + 128]
                                for f in range(2):
                                    nc.tensor.matmul(
                                        ps1[f][:, ci * 512 + 128 * h:
                                               ci * 512 + 128 * h + 130],
                                        lhsT=lhsT,
                                        rhs=Wt1[:, ((f * 2 + h) * 2 + j) * 130:
                                                ((f * 2 + h) * 2 + j + 1) * 130],
                                        start=(h == 0 and j == 0),
                                        stop=(h == 1 and j == 1),
                                    )
                    for f in range(2):
                        src = ps1[f][:].rearrange("p (b x) -> p b x", b=2)[:, :, 0:OUT]
                        dst = Yt[:, (f * 4 + 2 * ccp) * OUT:
                                 (f * 4 + 2 * ccp + 2) * OUT].rearrange(
                                     "p (b k) -> p b k", b=2)
                        copy(dst, src)

                # prefetch next image's input
                if i + 1 < IMGS:
                    Xnext = load_x(i + 1)

                # pass 2: STG blocks per STG_BLOCKS order
                STG = spool.tile([128, 10 * OUT], BF16, tag="STG", name=f"STG_{i}")
                t5 = pspool.tile([128, 1024], F32, tag="ps2", bufs=PS2_BUFS, name=f"t5_{i}")

                def mm_chunk(dst_ap, f, C):
                    ccs = CHUNK_CC[C]
                    for a, cc in enumerate(ccs):
                        s_idx = W2_SLICES.index((C, cc))
                        nc.tensor.matmul(
                            dst_ap,
                            lhsT=Wt2[:, s_idx * 128:(s_idx + 1) * 128],
                            rhs=Yt[:, (f * 4 + cc) * OUT:(f * 4 + cc + 1) * OUT],
                            start=(a == 0),
                            stop=(a == len(ccs) - 1),
                        )

                for f in range(2):
                    tA = pspool.tile([128, 1024], F32, tag="ps2", bufs=PS2_BUFS, name=f"tA_{i}_{f}")
                    mm_chunk(tA[:, 0:OUT], f, 0)
                    mm_chunk(tA[:, 512:512 + OUT], f, 1)
                    copy(
                        STG[:, (f * 4) * OUT:(f * 4 + 2) * OUT].rearrange(
                            "p (b k) -> p b k", b=2),
                        tA[:].rearrange("p (b x) -> p b x", b=2)[:, :, 0:OUT],
                    )
                    tB = pspool.tile([128, 1024], F32, tag="ps2", bufs=PS2_BUFS, name=f"tB_{i}_{f}")
                    mm_chunk(tB[:, 0:OUT], f, 2)
                    mm_chunk(tB[:, 512:512 + OUT], f, 3)
                    copy(
                        STG[:, (f * 4 + 2) * OUT:(f * 4 + 4) * OUT].rearrange(
                            "p (b k) -> p b k", b=2),
                        tB[:].rearrange("p (b x) -> p b x", b=2)[:, :, 0:OUT],
                    )
                    mm_chunk(t5[:, f * 512:f * 512 + OUT], f, 4)
                copy(
                    STG[:, 8 * OUT:10 * OUT].rearrange("p (b k) -> p b k", b=2),
                    t5[:].rearrange("p (b x) -> p b x", b=2)[:, :, 0:OUT],
                )

                nc.gpsimd.dma_start(out=y_out[i], in_=STG[:])
    nc.finalize()
    return nc


def _get_module() -> bass.Bass:
    global _MODULE
    if _MODULE is None:
        _MODULE = _build_module()
    return _MODULE


def _make_in_maps(x: np.ndarray) -> list:
    imgs = x.reshape(N_CORES * IMGS, H, W).astype(NPBF16)
    return [
        {"x": imgs[k * IMGS:(k + 1) * IMGS], "w1": _W1, "w2": _W2}
        for k in range(N_CORES)
    ]


def _unpack(y: np.ndarray, B: int, C: int) -> np.ndarray:
    """y: [n_imgs, 128, 10*258] bf16 -> [B, 4C, 258, 258] fp32."""
    n = y.shape[0]
    y = y.astype(np.float32).reshape(n, 128, 10, OUT)
    full = np.empty((n, 4, OUT, OUT), np.float32)
    for b, (f, Ck) in enumerate(STG_BLOCKS):
        blk = y[:, :, b, :]  # [n, 128(g,kw), 258(kh)]
        for g in range(2):
            s = f + 2 * g
            sel = blk[:, g * 64:(g + 1) * 64, :]
            kws = KWSTART[Ck] + np.arange(64)
            if Ck == 3:
                pass  # kw 192..255, all valid
            elif Ck == 4:
                sel = sel[:, 62:, :]  # only kw 256, 257
                kws = kws[62:]
            full[:, s, :, kws[0]:kws[-1] + 1] = sel.transpose(0, 2, 1)
    return np.ascontiguousarray(full.reshape(B, 4 * C, OUT, OUT))


def kernel(**inputs) -> np.ndarray:
    x = np.asarray(inputs["x"], dtype=np.float32)
    B, C, Hx, Wx = x.shape
    assert (Hx, Wx) == (H, W) and B * C == N_CORES * IMGS

    nc = _get_module()
    res = run_bass_kernel_spmd(nc, _make_in_maps(x), list(range(N_CORES))).results
    y = np.concatenate([res[k]["y"] for k in range(N_CORES)], axis=0)
    return _unpack(y, B, C)


# revision 7
# speedup vs baseline: 6.0948x; 1.1227x over previous
"""Trainium2 Bass kernel for 2D single-level DWT (coif1, symmetric padding).

Input  x: (4, 64, 512, 512) fp32
Output  : (4, 256, 258, 258) fp32  -- per input channel: [cA, cH, cV, cD]

v2 design (bf16, banded half-blocks, SWDGE stores):
  pass 1 (contract rows r): r is split into half-blocks h in {0,1} of 256
    contiguous rows, each loaded as partition p <- rows (256h + 2p + j),
    j in {0,1} -- every DMA descriptor is 2 contiguous DRAM rows (2 KB).
    The 6-tap band of R_f means half-block h only feeds kh in
    [128h, 128h+130), so each matmul streams only 130 columns; the 2-col
    overlap accumulates via PSUM has_written semantics.
      Yt_f[c, kh] = sum_r X[r, c] R_f[kh, r]
  pass 2 (contract cols c): output rows (g, kw) are packed in uniform
    128-partition chunks of 64 kw x {lo,hi}: chunks start at kw =
    0, 64, 128, 192, 194 (the last overlaps; host keeps only kw 256-257
    from it).  Each chunk's band covers at most 2 c-blocks of 128 -> 1-2
    accumulating matmuls of 258 columns.
      O_{f,g}[kw, kh] = sum_c R_g[kw, c] Yt_f[c, kh]
  All matmuls bf16 (1 cycle/row at any free size, FWL weight loads).
  Loads and stores ride the gpsimd SWDGE ring: HWDGE stores with <128
  partitions serialize onto 2 of 16 SDMA engines; SWDGE spreads all 16.
  Output leaves packed ([i, p, block, kh], bf16); the host unpacks.
"""

import os
import sys

for _p in ("/opt/trn_rl_repo", "/opt/pypackages"):
    if _p not in sys.path:
        sys.path.append(_p)

os.environ.setdefault("JAX_COMPILATION_CACHE_DIR", "/tmp/jax_comp_cache")
os.environ.setdefault("JAX_PERSISTENT_CACHE_MIN_COMPILE_TIME_SECS", "10")

import numpy as np
import ml_dtypes

import concourse.bass as bass
import concourse.bacc as bacc
import concourse.mybir as mybir
from concourse.bass_utils import run_bass_kernel_spmd
from concourse.tile import TileContext

N_CORES = 8
H = W = 512
OUT = 258
IMGS = 32  # images per core (4*64/8)
F32 = mybir.dt.float32
BF16 = mybir.dt.bfloat16
NPBF16 = ml_dtypes.bfloat16

# pywt coif1 decomposition filters, flipped to correlation form
DEC_LO = np.array([-0.01565572813546454, -0.0727326195128539, 0.38486484686420286,
                   0.8525720202122554, 0.3378976624578092, -0.0727326195128539])
DEC_HI = np.array([0.0727326195128539, 0.3378976624578092, -0.8525720202122554,
                   0.38486484686420286, 0.0727326195128539, -0.01565572813546454])
FLEN = 6
PAD = 4
LO_F = DEC_LO[::-1]
HI_F = DEC_HI[::-1]

# pass-2 packed chunks: 64 kw starting at KWSTART[C]; CHUNK_CC[C] = c-blocks
KWSTART = [0, 64, 128, 192, 194]
CHUNK_CC = [[0], [0, 1], [1, 2], [2, 3], [3]]
W2_SLICES = [(C, cc) for C in range(5) for cc in CHUNK_CC[C]]  # 8 slices
# STG block order = device copy order (tA_f0, tB_f0, tA_f1, tB_f1, t5)
STG_BLOCKS = [(0, 0), (0, 1), (0, 2), (0, 3),
              (1, 0), (1, 1), (1, 2), (1, 3),
              (0, 4), (1, 4)]


def _build_R(filt: np.ndarray, n: int = W) -> np.ndarray:
    """Banded [258, 512] operator: out[k] = sum_j filt[j] * x[sym(2k + j - PAD)]."""
    out_len = (n + FLEN - 1) // 2

    def sym(i: int) -> int:
        while i < 0 or i >= n:
            if i < 0:
                i = -i - 1
            if i >= n:
                i = 2 * n - 1 - i
        return i

    R = np.zeros((out_len, n), dtype=np.float64)
    for k in range(out_len):
        for j in range(FLEN):
            R[k, sym(2 * k + j - PAD)] += filt[j]
    return R


_R = [_build_R(LO_F), _build_R(HI_F)]


def _build_w1() -> np.ndarray:
    """w1[p, ((f*2+h)*2+j)*130 + t] = R_f[128h + t, 256h + 2p + j]."""
    w = np.zeros((128, 8, 130), np.float64)
    for f in range(2):
        for h in range(2):
            for j in range(2):
                rows = 256 * h + 2 * np.arange(128) + j
                khs = 128 * h + np.arange(130)
                w[:, (f * 2 + h) * 2 + j, :] = _R[f][np.ix_(khs, rows)].T
    return w.reshape(128, 8 * 130).astype(NPBF16)


def _build_w2() -> np.ndarray:
    """w2[p, s*128 + u] for slice s=(C, cc): R_{u//64}[KWSTART[C] + u%64, 128cc + p]."""
    cols = []
    for C, cc in W2_SLICES:
        w = np.zeros((128, 128), np.float64)
        for u in range(128):
            g, kwo = divmod(u, 64)
            w[:, u] = _R[g][KWSTART[C] + kwo, cc * 128:(cc + 1) * 128]
        cols.append(w)
    return np.concatenate(cols, axis=1).astype(NPBF16)


_W1 = _build_w1()
_W2 = _build_w2()
_MODULE = None
PS_BUFS = 4
X_BUFS = 3
YT_BUFS = 2
STG_BUFS = 2


def _build_module() -> bass.Bass:
    nc = bacc.Bacc("TRN2", target_bir_lowering=False, debug=False)
    x_in = nc.declare_dram_parameter("x", [IMGS, H, W], BF16, isOutput=False)
    w1_in = nc.declare_dram_parameter("w1", [128, 8 * 130], BF16, isOutput=False)
    w2_in = nc.declare_dram_parameter("w2", [128, 8 * 128], BF16, isOutput=False)
    y_out = nc.declare_dram_parameter("y", [IMGS, 128, 10 * OUT], BF16, isOutput=True)

    with TileContext(nc) as tc:
        with (
            tc.tile_pool(name="wpool", bufs=1) as wpool,
            tc.tile_pool(name="xpool", bufs=X_BUFS) as xpool,
            tc.tile_pool(name="ypool", bufs=YT_BUFS) as ypool,
            tc.tile_pool(name="spool", bufs=STG_BUFS) as spool,
            tc.tile_pool(name="psum", bufs=2, space="PSUM") as pspool,
        ):
            Wt1 = wpool.tile([128, 8 * 130], BF16)
            Wt2 = wpool.tile([128, 8 * 128], BF16)
            nc.sync.dma_start(out=Wt1[:], in_=w1_in[:])
            nc.sync.dma_start(out=Wt2[:], in_=w2_in[:])

            # Tiny PE op consuming both weight DMAs so later matmuls depend
            # on them via PE program order (Matmult carries one sync wait).
            warm = pspool.tile([128, 1024], F32, tag="ps", bufs=PS_BUFS,
                               name="warm")
            nc.tensor.matmul(warm[0:1, 0:OUT], lhsT=Wt2[:, 0:1],
                             rhs=Wt1[:, 0:OUT], start=True, stop=True)

            ev = 0

            def copy(dst, src):
                nonlocal ev
                if ev % 2 == 0:
                    nc.scalar.copy(out=dst, in_=src)
                else:
                    nc.vector.tensor_copy(out=dst, in_=src)
                ev += 1

            def load_x(i):
                # X[p, h, j*512 + c] = x[i, 256h + 2p + j, c]
                X = xpool.tile([128, 2, 1024], BF16, tag="X", name=f"X_{i}")
                nc.gpsimd.dma_start(
                    out=X[:],
                    in_=x_in[i].rearrange("(h p j) c -> p h (j c)", h=2, j=2),
                )
                return X

            Xnext = load_x(0)
            for i in range(IMGS):
                Xr = Xnext[:]
                Yt = ypool.tile([128, 8 * OUT], BF16, tag="Yt", name=f"Yt_{i}")

                # pass 1: Yt[p, (f*4+cc)*258 + kh] = Yt_f[c = 128cc + p, kh]
                for ccp in range(2):  # cc pair (2*ccp, 2*ccp+1)
                    ps1 = [pspool.tile([128, 1024], F32, tag="ps", bufs=PS_BUFS,
                                       name=f"ps1_{i}_{ccp}_{f}")
                           for f in range(2)]
                    for ci in range(2):
                        cc = 2 * ccp + ci
                        for h in range(2):
                            for j in range(2):
                                lhsT = Xr[:, h, j * 512 + cc * 128:
                                          j * 512 + cc * 128 + 128]
                                for f in range(2):
                                    nc.tensor.matmul(
                                        ps1[f][:, ci * 512 + 128 * h:
                                               ci * 512 + 128 * h + 130],
                                        lhsT=lhsT,
                                        rhs=Wt1[:, ((f * 2 + h) * 2 + j) * 130:
                                                ((f * 2 + h) * 2 + j + 1) * 130],
                                        start=(h == 0 and j == 0),
                                        stop=(h == 1 and j == 1),
                                    )
                    for f in range(2):
                        src = ps1[f][:].rearrange("p (b x) -> p b x", b=2)[:, :, 0:OUT]
                        dst = Yt[:, (f * 4 + 2 * ccp) * OUT:
                                 (f * 4 + 2 * ccp + 2) * OUT].rearrange(
                                     "p (b k) -> p b k", b=2)
                        copy(dst, src)

                # prefetch next image's input
                if i + 1 < IMGS:
                    Xnext = load_x(i + 1)

                # pass 2: STG blocks per STG_BLOCKS order
                STG = spool.tile([128, 10 * OUT], BF16, tag="STG", name=f"STG_{i}")

                def mm_chunk(dst_ap, f, C):
                    ccs = CHUNK_CC[C]
                    for a, cc in enumerate(ccs):
                        s_idx = W2_SLICES.index((C, cc))
                        nc.tensor.matmul(
                            dst_ap,
                            lhsT=Wt2[:, s_idx * 128:(s_idx + 1) * 128],
                            rhs=Yt[:, (f * 4 + cc) * OUT:(f * 4 + cc + 1) * OUT],
                            start=(a == 0),
                            stop=(a == len(ccs) - 1),
                        )

                for f in range(2):
                    tA = pspool.tile([128, 1024], F32, tag="ps", bufs=PS_BUFS, name=f"tA_{i}_{f}")
                    mm_chunk(tA[:, 0:OUT], f, 0)
                    mm_chunk(tA[:, 512:512 + OUT], f, 1)
                    copy(
                        STG[:, (f * 4) * OUT:(f * 4 + 2) * OUT].rearrange(
                            "p (b k) -> p b k", b=2),
                        tA[:].rearrange("p (b x) -> p b x", b=2)[:, :, 0:OUT],
                    )
                    tB = pspool.tile([128, 1024], F32, tag="ps", bufs=PS_BUFS, name=f"tB_{i}_{f}")
                    mm_chunk(tB[:, 0:OUT], f, 2)
                    mm_chunk(tB[:, 512:512 + OUT], f, 3)
                    copy(
                        STG[:, (f * 4 + 2) * OUT:(f * 4 + 4) * OUT].rearrange(
                            "p (b k) -> p b k", b=2),
                        tB[:].rearrange("p (b x) -> p b x", b=2)[:, :, 0:OUT],
                    )
                t5 = pspool.tile([128, 1024], F32, tag="ps", bufs=PS_BUFS, name=f"t5_{i}")
                mm_chunk(t5[:, 0:OUT], 0, 4)
                mm_chunk(t5[:, 512:512 + OUT], 1, 4)
                copy(
                    STG[:, 8 * OUT:10 * OUT].rearrange("p (b k) -> p b k", b=2),
                    t5[:].rearrange("p (b x) -> p b x", b=2)[:, :, 0:OUT],
                )

                nc.gpsimd.dma_start(out=y_out[i], in_=STG[:])
    nc.finalize()
    return nc


def _get_module() -> bass.Bass:
    global _MODULE
    if _MODULE is None:
        _MODULE = _build_module()
    return _MODULE


def _make_in_maps(x: np.ndarray) -> list:
    imgs = x.reshape(N_CORES * IMGS, H, W).astype(NPBF16)
    return [
        {"x": imgs[k * IMGS:(k + 1) * IMGS], "w1": _W1, "w2": _W2}
        for k in range(N_CORES)
    ]


def _unpack(y: np.ndarray, B: int, C: int) -> np.ndarray:
    """y: [n_imgs, 128, 10*258] bf16 -> [B, 4C, 258, 258] fp32."""
    n = y.shape[0]
    y = y.astype(np.float32).reshape(n, 128, 10, OUT)
    full = np.empty((n, 4, OUT, OUT), np.float32)
    for b, (f, Ck) in enumerate(STG_BLOCKS):
        blk = y[:, :, b, :]  # [n, 128(g,kw), 258(kh)]
        for g in range(2):
            s = f + 2 * g
            sel = blk[:, g * 64:(g + 1) * 64, :]
            kws = KWSTART[Ck] + np.arange(64)
            if Ck == 3:
                pass  # kw 192..255, all valid
            elif Ck == 4:
                sel = sel[:, 62:, :]  # only kw 256, 257
                kws = kws[62:]
            full[:, s, :, kws[0]:kws[-1] + 1] = sel.transpose(0, 2, 1)
    return np.ascontiguousarray(full.reshape(B, 4 * C, OUT, OUT))


def kernel(**inputs) -> np.ndarray:
    x = np.asarray(inputs["x"], dtype=np.float32)
    B, C, Hx, Wx = x.shape
    assert (Hx, Wx) == (H, W) and B * C == N_CORES * IMGS

    nc = _get_module()
    res = run_bass_kernel_spmd(nc, _make_in_maps(x), list(range(N_CORES))).results
    y = np.concatenate([res[k]["y"] for k in range(N_CORES)], axis=0)
    return _unpack(y, B, C)
